# revision 18
# baseline (speedup 1.0000x reference)
"""Trainium2 Bass kernel for CausalRecurrenceLayer.

Sharding: 8 cores = B(4) x L-halves(2). Each core processes a 4096-token
half-sequence with all 1024 channels in [channel, time] layout.

Two SPMD launches:
  A: causal depthwise conv -> r/i gate matmuls (fp32r) -> a_t/b_t ->
     local scan h_l = scan(a, b, init=0) and A = cumprod(a).
  host: carry fix-up h_g = h_l + A * h_even[-1] for the second halves.
  B: y = h_g @ out_w.T (fp32r) -> rmsnorm -> out.
"""
import numpy as np
from contextlib import ExitStack

import concourse.bass as bass
import concourse.tile as tile
from concourse import bacc, mybir
from concourse.bass_utils import run_bass_kernel_spmd

F32 = mybir.dt.float32
F32R = mybir.dt.float32r
BF16 = mybir.dt.bfloat16
AF = mybir.ActivationFunctionType
OP = mybir.AluOpType

D = 1024
KCONV = 4
TCHUNK = 4096          # tokens per core
TT = 512               # t-tile
NT = TCHUNK // TT      # 8 t-tiles
NC = D // 128          # 8 channel chunks

_CACHE = {}


def _build_launch_a():
    nc = bacc.Bacc("TRN2", target_bir_lowering=False, debug=False, num_devices=8)
    xt = nc.dram_tensor("xt", [D, TCHUNK + KCONV - 1], F32, kind="ExternalInput")
    wrt = nc.dram_tensor("wrt", [D, D], BF16, kind="ExternalInput")
    wit = nc.dram_tensor("wit", [D, D], BF16, kind="ExternalInput")
    cw = nc.dram_tensor("cw", [128, NC * KCONV], F32, kind="ExternalInput")
    cb = nc.dram_tensor("cb", [128, NC], F32, kind="ExternalInput")
    rb = nc.dram_tensor("rb", [128, NC], F32, kind="ExternalInput")
    ib = nc.dram_tensor("ib", [128, NC], F32, kind="ExternalInput")
    la8 = nc.dram_tensor("la8", [128, NC], F32, kind="ExternalInput")
    la16 = nc.dram_tensor("la16", [128, NC], F32, kind="ExternalInput")
    hl = nc.dram_tensor("hl", [D, TCHUNK], F32, kind="ExternalOutput")
    aa = nc.dram_tensor("aa", [D, TCHUNK], F32, kind="ExternalOutput")

    with tile.TileContext(nc) as tc, ExitStack() as ctx:
        consts = ctx.enter_context(tc.tile_pool(name="consts", bufs=1))
        wpool = ctx.enter_context(tc.tile_pool(name="weights", bufs=1))
        xv_p = ctx.enter_context(tc.tile_pool(name="xv", bufs=6))
        xc_p = ctx.enter_context(tc.tile_pool(name="xc", bufs=12))
        sr_p = ctx.enter_context(tc.tile_pool(name="sr", bufs=4))
        si_p = ctx.enter_context(tc.tile_pool(name="si", bufs=4))
        a_p = ctx.enter_context(tc.tile_pool(name="a", bufs=5))
        a2_p = ctx.enter_context(tc.tile_pool(name="a2", bufs=4))
        q_p = ctx.enter_context(tc.tile_pool(name="q", bufs=4))
        u_p = ctx.enter_context(tc.tile_pool(name="u", bufs=4))
        b_p = ctx.enter_context(tc.tile_pool(name="b", bufs=5))
        h_p = ctx.enter_context(tc.tile_pool(name="h", bufs=4))
        A_p = ctx.enter_context(tc.tile_pool(name="A", bufs=4))
        psum = ctx.enter_context(tc.tile_pool(name="psum", bufs=4, space="PSUM"))

        # resident weights: wrt/wit as 8 chunks of [128, D]
        wr_sb, wi_sb = [], []
        for kc in range(NC):
            t1 = wpool.tile([128, D], BF16, tag=f"wr{kc}")
            nc.sync.dma_start(t1[:], wrt[kc * 128:(kc + 1) * 128, :])
            wr_sb.append(t1)
            t2 = wpool.tile([128, D], BF16, tag=f"wi{kc}")
            nc.sync.dma_start(t2[:], wit[kc * 128:(kc + 1) * 128, :])
            wi_sb.append(t2)
        cw_t = consts.tile([128, NC * KCONV], F32)
        nc.sync.dma_start(cw_t[:], cw[:])
        cb_t = consts.tile([128, NC], F32)
        nc.sync.dma_start(cb_t[:], cb[:])
        rb_t = consts.tile([128, NC], F32)
        nc.sync.dma_start(rb_t[:], rb[:])
        ib_t = consts.tile([128, NC], F32)
        nc.sync.dma_start(ib_t[:], ib[:])
        la8_t = consts.tile([128, NC], F32)
        nc.sync.dma_start(la8_t[:], la8[:])
        la16_t = consts.tile([128, NC], F32)
        nc.sync.dma_start(la16_t[:], la16[:])
        zeros = consts.tile([128, TT], F32)
        nc.vector.memset(zeros[:], 0.0)
        # persistent carry columns: scan initials for each channel chunk
        hlast = consts.tile([128, NC], F32)
        nc.vector.memset(hlast[:], 0.0)
        Alast = consts.tile([128, NC], F32)
        nc.vector.memset(Alast[:], 1.0)
        for it in range(NT):
            t0 = it * TT
            xc = []
            for cc in range(NC):
                xv = xv_p.tile([128, TT + KCONV - 1], F32, tag="xv")
                nc.sync.dma_start(
                    xv[:], xt[cc * 128:(cc + 1) * 128, t0:t0 + TT + KCONV - 1]
                )
                xct = xc_p.tile([128, TT], F32, tag="xc")
                w = lambda k: cw_t[:, cc * KCONV + k:cc * KCONV + k + 1]
                nc.scalar.activation(xct[:], xv[:, 0:TT], AF.Copy, bias=0.0, scale=w(0))
                for k in range(1, KCONV):
                    nc.vector.scalar_tensor_tensor(
                        xct[:], xv[:, k:k + TT], w(k), xct[:], OP.mult, OP.add
                    )
                xc.append(xct)
            xcb = []
            for cc in range(NC):
                t16 = xc_p.tile([128, TT], BF16, tag="xcb")
                nc.scalar.copy(t16[:], xc[cc][:])
                xcb.append(t16)
            for jc in range(NC):
                zr = psum.tile([128, TT], F32, tag="z")
                for kc in range(NC):
                    nc.tensor.matmul(
                        zr[:],
                        wr_sb[kc][:, jc * 128:(jc + 1) * 128],
                        xcb[kc][:],
                        start=(kc == 0), stop=(kc == NC - 1),
                    )
                sr = sr_p.tile([128, TT], F32, tag="sr")
                nc.scalar.activation(
                    sr[:], zr[:], AF.Sigmoid, bias=rb_t[:, jc:jc + 1]
                )
                zi = psum.tile([128, TT], F32, tag="z")
                for kc in range(NC):
                    nc.tensor.matmul(
                        zi[:],
                        wi_sb[kc][:, jc * 128:(jc + 1) * 128],
                        xcb[kc][:],
                        start=(kc == 0), stop=(kc == NC - 1),
                    )
                si = si_p.tile([128, TT], F32, tag="si")
                nc.scalar.activation(
                    si[:], zi[:], AF.Sigmoid, bias=ib_t[:, jc:jc + 1]
                )
                at = a_p.tile([128, TT], F32, tag="a")
                nc.scalar.activation(
                    at[:], sr[:], AF.Exp, bias=0.0, scale=la8_t[:, jc:jc + 1]
                )
                a2t = a2_p.tile([128, TT], F32, tag="a2")
                nc.scalar.activation(
                    a2t[:], sr[:], AF.Exp, bias=0.0, scale=la16_t[:, jc:jc + 1]
                )
                qt = q_p.tile([128, TT], F32, tag="q")
                nc.scalar.activation(qt[:], a2t[:], AF.Sqrt, bias=1.0, scale=-1.0)
                ut = u_p.tile([128, TT], F32, tag="u")
                nc.vector.scalar_tensor_tensor(
                    ut[:], xc[jc][:], cb_t[:, jc:jc + 1], si[:], OP.add, OP.mult
                )
                bt = b_p.tile([128, TT], F32, tag="b")
                nc.vector.tensor_tensor(bt[:], qt[:], ut[:], OP.mult)
                ht = h_p.tile([128, TT], F32, tag="h")
                nc.vector.tensor_tensor_scan(
                    ht[:], at[:], bt[:], hlast[:, jc:jc + 1], OP.mult, OP.add
                )
                nc.scalar.copy(hlast[:, jc:jc + 1], ht[:, TT - 1:TT])
                At = A_p.tile([128, TT], F32, tag="A")
                nc.vector.tensor_tensor_scan(
                    At[:], at[:], zeros[:], Alast[:, jc:jc + 1], OP.mult, OP.add
                )
                nc.scalar.copy(Alast[:, jc:jc + 1], At[:, TT - 1:TT])
                nc.sync.dma_start(hl[jc * 128:(jc + 1) * 128, t0:t0 + TT], ht[:])
                nc.sync.dma_start(aa[jc * 128:(jc + 1) * 128, t0:t0 + TT], At[:])
    nc.compile()
    return nc


def _build_launch_b():
    nc = bacc.Bacc("TRN2", target_bir_lowering=False, debug=False, num_devices=8)
    hg = nc.dram_tensor("hg", [D, TCHUNK], F32R, kind="ExternalInput")
    owt = nc.dram_tensor("owt", [D, D], F32R, kind="ExternalInput")
    nwb = nc.dram_tensor("nwb", [128, D], F32, kind="ExternalInput")
    out = nc.dram_tensor("out", [TCHUNK, D], F32, kind="ExternalOutput")

    NG = TCHUNK // 128  # 32 token groups
    with tile.TileContext(nc) as tc, ExitStack() as ctx:
        consts = ctx.enter_context(tc.tile_pool(name="consts", bufs=1))
        wpool = ctx.enter_context(tc.tile_pool(name="weights", bufs=1))
        h_p = ctx.enter_context(tc.tile_pool(name="h", bufs=24))
        o_p = ctx.enter_context(tc.tile_pool(name="o", bufs=3))
        s_p = ctx.enter_context(tc.tile_pool(name="s", bufs=3))
        v_p = ctx.enter_context(tc.tile_pool(name="v", bufs=4))
        psum = ctx.enter_context(tc.tile_pool(name="psum", bufs=3, space="PSUM"))

        ow_sb = []
        for kc in range(NC):
            t1 = wpool.tile([128, D], F32R, tag=f"ow{kc}")
            nc.sync.dma_start(t1[:], owt[kc * 128:(kc + 1) * 128, :])
            ow_sb.append(t1)
        nw_t = consts.tile([128, D], F32)
        nc.sync.dma_start(nw_t[:], nwb[:])
        eps_t = consts.tile([128, 1], F32)
        nc.vector.memset(eps_t[:], 1e-6)

        for g in range(NG):
            t0 = g * 128
            hts = []
            for kc in range(NC):
                ht = h_p.tile([128, 128], F32R, tag="h")
                nc.sync.dma_start(ht[:], hg[kc * 128:(kc + 1) * 128, t0:t0 + 128])
                hts.append(ht)
            y = psum.tile([128, D], F32, tag="y")
            for jh in range(2):
                for kc in range(NC):
                    nc.tensor.matmul(
                        y[:, jh * 512:(jh + 1) * 512],
                        hts[kc][:],
                        ow_sb[kc][:, jh * 512:(jh + 1) * 512],
                        start=(kc == 0), stop=(kc == NC - 1),
                    )
            sq = s_p.tile([128, D], F32, tag="sq")
            ss = v_p.tile([128, 1], F32, tag="ss")
            nc.scalar.activation(sq[:], y[:], AF.Square, accum_out=ss[:])
            srt = v_p.tile([128, 1], F32, tag="srt")
            nc.scalar.activation(
                srt[:], ss[:], AF.Sqrt, bias=eps_t[:], scale=1.0 / D
            )
            rr = v_p.tile([128, 1], F32, tag="rr")
            nc.vector.reciprocal(rr[:], srt[:])
            ot = o_p.tile([128, D], F32, tag="o")
            nc.vector.scalar_tensor_tensor(
                ot[:], y[:], rr[:], nw_t[:], OP.mult, OP.mult
            )
            nc.sync.dma_start(out[t0:t0 + 128, :], ot[:])
    nc.compile()
    return nc


def kernel(x, conv_weight, conv_bias, Wr_w, Wr_b, Wi_w, Wi_b, log_a, out_w, norm_w):
    B, L, d = x.shape
    assert (B, L, d) == (4, 8192, D)
    half = L // 2

    # host-side prep (fp32)
    x = np.ascontiguousarray(x, dtype=np.float32)
    la = np.log(1.0 / (1.0 + np.exp(-log_a.astype(np.float64))))
    la8 = (8.0 * la).astype(np.float32)
    la16 = (16.0 * la).astype(np.float32)
    rb_f = (Wr_b.astype(np.float64)
            + conv_bias.astype(np.float64) @ Wr_w.T.astype(np.float64)
            ).astype(np.float32)
    ib_f = (Wi_b.astype(np.float64)
            + conv_bias.astype(np.float64) @ Wi_w.T.astype(np.float64)
            ).astype(np.float32)

    def pack(v):  # [D] -> [128, NC]
        return np.ascontiguousarray(v.reshape(NC, 128).T, dtype=np.float32)

    cw_pk = np.ascontiguousarray(
        conv_weight[:, 0, :].reshape(NC, 128, KCONV).transpose(1, 0, 2)
        .reshape(128, NC * KCONV), dtype=np.float32)
    import ml_dtypes
    cb_pk, rb_pk, ib_pk = pack(conv_bias), pack(rb_f), pack(ib_f)
    la8_pk, la16_pk = pack(la8), pack(la16)
    wrt = np.ascontiguousarray(Wr_w.T.astype(ml_dtypes.bfloat16))
    wit = np.ascontiguousarray(Wi_w.T.astype(ml_dtypes.bfloat16))
    owt = np.ascontiguousarray(out_w.T, dtype=np.float32)
    nwb = np.ascontiguousarray(
        np.broadcast_to(norm_w.astype(np.float32)[None, :], (128, D)))

    if "A" not in _CACHE:
        _CACHE["A"] = _build_launch_a()
    ncA = _CACHE["A"]
    in_maps_a = []
    for core in range(8):
        b, hf = core // 2, core % 2
        t0 = hf * half
        xt = np.zeros((D, half + KCONV - 1), np.float32)
        seg = x[b, max(0, t0 - (KCONV - 1)):t0 + half, :]
        xt[:, (KCONV - 1) - (t0 - max(0, t0 - (KCONV - 1))):] = seg.T
        in_maps_a.append({
            "xt": xt, "wrt": wrt, "wit": wit, "cw": cw_pk, "cb": cb_pk,
            "rb": rb_pk, "ib": ib_pk, "la8": la8_pk, "la16": la16_pk,
        })
    resA = run_bass_kernel_spmd(ncA, in_maps_a, core_ids=list(range(8)))

    # host carry fix-up
    hgs = []
    for core in range(8):
        hl = resA.results[core]["hl"]
        if core % 2 == 0:
            hgs.append(hl)
        else:
            carry = resA.results[core - 1]["hl"][:, -1:]
            hgs.append(hl + resA.results[core]["aa"] * carry)

    if "B" not in _CACHE:
        _CACHE["B"] = _build_launch_b()
    ncB = _CACHE["B"]
    in_maps_b = [{"hg": np.ascontiguousarray(hgs[c]), "owt": owt, "nwb": nwb}
                 for c in range(8)]
    resB = run_bass_kernel_spmd(ncB, in_maps_b, core_ids=list(range(8)))

    out = np.empty((B, L, D), np.float32)
    for core in range(8):
        b, hf = core // 2, core % 2
        out[b, hf * half:(hf + 1) * half, :] = resB.results[core]["out"]
    return out
